# revision 21
# baseline (speedup 1.0000x reference)
"""Trainium2 Bass kernel for a causal single-head attention block.

Reference computation (fp32):
    q = x @ Wq; k = x @ Wk; v = x @ Wv        x: [B=256, T=256, C=384], W*: [384, 64]
    wei = softmax(causal_mask(q @ k.T / sqrt(C)))
    out = wei @ v                              out: [256, 256, 64]

Strategy: pure data parallel over B across 8 NeuronCores (32 batches/core).

All matmul operands are fp16: 2-byte operands stream through the PE at full
rate (4-byte fp32/fp32r streams at half rate), get fast weight load, and allow
the DMA engines' 2-byte transpose XBAR. PSUM accumulation stays fp32; measured
end-to-end error vs the fp32 reference is ~8e-4 (scale-relative).

Per-batch dataflow:
    xT    = DMA-transpose(x_b)                 [C, T] fp16 sbuf [128, 768], 3 DMAs,
                                               no PE/DVE involvement at all
    qT|kT = [Wq|Wk].T @ xT                     3 accumulating matmuls, M=128 packed
                                               (q on partitions 0-63, k on 64-127)
    kT    -> own tile via SBUF->SBUF DMA       (cross-partition move, scalar DGE)
    v     = xT.T @ Wv                          [128, 64+64] (both t tiles, one psum)
    sT    = kT[:, s_tile].T @ qT               scores transposed [s, t]
    P     = exp(sT * 1/sqrt(C)) * mask01       exp on ScalarE from psum, 0/1 mask
                                               multiply on DVE (fp16, 2x mode)
    oeT   = [v | 1].T @ P                      [65, 256] psum; row 64 = denominator Z
    out   = PE-transpose(oeT) * (1/Z)          normalize on ScalarE, DMA out (GpSimd
                                               DGE so the sync queue stays short)

Software pipeline (PE stream per iteration):
    sc(b) qkT(b+1) v(b+1) oe(b) finT(b-1)
so the softmax chain of batch b (ACT exp + DVE mask) runs under the next
batch's projections, and finT's oe-copy dependency is a full iteration old.
"""

import numpy as np

N_EMBED = 384
HEAD_SIZE = 64
H1 = HEAD_SIZE + 1
T = 256
B = 256
N_CORES = 8
B_SHARD = B // N_CORES  # 32
CC = N_EMBED // 128  # 3 contraction chunks
INV_SQRT_C = 1.0 / float(np.sqrt(N_EMBED))

_CACHE = {}

# test.py can flip these before calling kernel()
TRACE = False
LAST_RESULTS = None


def _build_program():
    import concourse.bacc as bacc
    import concourse.mybir as mybir
    import concourse.tile as tile
    from concourse import bass

    f32 = mybir.dt.float32
    f16 = mybir.dt.float16
    ts = bass.ts
    Exp = mybir.ActivationFunctionType.Exp
    Copy = mybir.ActivationFunctionType.Copy

    nc = bacc.Bacc("TRN2", target_bir_lowering=False, debug=False,
                   enable_asserts=False)

    x_d = nc.dram_tensor("x", [B_SHARD, T, N_EMBED], f16, kind="ExternalInput")
    wqk_d = nc.dram_tensor("Wqk", [CC, 128, 128], f16, kind="ExternalInput")
    wv_d = nc.dram_tensor("Wv", [N_EMBED, HEAD_SIZE], f16, kind="ExternalInput")
    ident_d = nc.dram_tensor("ident", [128, 128], f16, kind="ExternalInput")
    mask_d = nc.dram_tensor("mask01", [2, 128, T], f16, kind="ExternalInput")
    out_d = nc.dram_tensor("out", [B_SHARD, T, HEAD_SIZE], f32, kind="ExternalOutput")

    x_ap = x_d.ap()
    out_ap = out_d.ap()

    with tile.TileContext(nc) as tc:
        with (
            tc.tile_pool(name="const", bufs=1) as cpool,
            tc.tile_pool(name="xin", bufs=6) as xin_pool,
            tc.tile_pool(name="xt", bufs=2) as xt_pool,
            tc.tile_pool(name="proj", bufs=7) as proj_pool,
            tc.tile_pool(name="vext", bufs=1) as vext_pool,
            tc.tile_pool(name="soft", bufs=6) as soft_pool,
            tc.tile_pool(name="outp", bufs=8) as out_pool,
            tc.tile_pool(name="ps", bufs=5, space="PSUM") as ps_pool,
            tc.tile_pool(name="ps16", bufs=3, space="PSUM") as ps16_pool,
        ):
            # ---- constants ----
            ident = cpool.tile([128, 128], f16, tag="ident")
            nc.sync.dma_start(ident[:], ident_d.ap())
            wqk_sb, wv_sb = [], []
            for cc in range(CC):
                t_ = cpool.tile([128, 128], f16, tag=f"wqk{cc}")
                nc.sync.dma_start(t_[:], wqk_d.ap()[cc])
                wqk_sb.append(t_)
                t_ = cpool.tile([128, HEAD_SIZE], f16, tag=f"wv{cc}")
                nc.sync.dma_start(t_[:], wv_d.ap()[ts(cc, 128), :])
                wv_sb.append(t_)
            mask_sb = []
            for st in range(2):
                t_ = cpool.tile([128, T], f16, tag=f"mask{st}")
                nc.sync.dma_start(t_[:], mask_d.ap()[st, :, :])
                mask_sb.append(t_)
            ones_col = cpool.tile([128, 1], f16, tag="ones")
            nc.gpsimd.memset(ones_col[:], 1.0)

            def load_x(b):
                tiles = []
                for tt in range(2):
                    t_ = xin_pool.tile([128, N_EMBED], f16, tag="x_nat")
                    nc.sync.dma_start(t_[:], x_ap[b, ts(tt, 128), :])
                    tiles.append(t_)
                return tiles

            def transpose_x(x_nat):
                """-> one [128, 768] fp16 sbuf tile; chunk cc at [:, cc*256:(cc+1)*256]"""
                ps = ps16_pool.tile([128, 3 * T], f16, tag="ps16")
                for cc in range(CC):
                    for tt in range(2):
                        nc.tensor.transpose(
                            ps[:, cc * T + tt * 128: cc * T + (tt + 1) * 128],
                            x_nat[tt][:, ts(cc, 128)], ident[:])
                sb = xt_pool.tile([128, 3 * T], f16, tag="xt")
                nc.vector.tensor_copy(sb[:], ps[:])
                return sb

            def qkT_mm(xt):
                """q and k projections in one M=128 stack; kT moved to its own
                base-0 tile with a cross-partition SBUF->SBUF DMA."""
                ps = ps_pool.tile([128, 512], f32, tag="ps")
                for cc in range(CC):
                    nc.tensor.matmul(ps[:, :T], wqk_sb[cc][:], xt[:, ts(cc, T)],
                                     start=(cc == 0), stop=(cc == CC - 1))
                sb = proj_pool.tile([128, T], f16, tag="qk")
                nc.vector.tensor_copy(sb[:], ps[:, :T])
                kT = proj_pool.tile([HEAD_SIZE, T], f16, tag="kT")
                nc.sync.dma_start(kT[:], sb[HEAD_SIZE:128, :])
                return sb, kT  # qT = sb[:64]

            def v_mm(xt, b):
                ps = ps_pool.tile([128, 512], f32, tag="ps")
                for tt in range(2):
                    for cc in range(CC):
                        nc.tensor.matmul(ps[:, tt * 128:tt * 128 + HEAD_SIZE],
                                         xt[:, cc * T + tt * 128: cc * T + (tt + 1) * 128],
                                         wv_sb[cc][:],
                                         start=(cc == 0), stop=(cc == CC - 1))
                v_ext = []
                for tt in range(2):
                    slot = (2 * b + tt) % 8
                    sb = vext_pool.tile([128, H1], f16, tag=f"vx{slot}")
                    nc.vector.tensor_copy(sb[:, :HEAD_SIZE],
                                          ps[:, tt * 128:tt * 128 + HEAD_SIZE])
                    if 2 * b + tt < 8:
                        # ones column persists in the slot across reuses
                        nc.vector.tensor_copy(sb[:, HEAD_SIZE:H1], ones_col[:])
                    v_ext.append(sb)
                return v_ext

            def scores_mm(qk, kT):
                ps = ps_pool.tile([128, 512], f32, tag="ps")
                for st in range(2):
                    nc.tensor.matmul(ps[:, ts(st, T)], kT[:, ts(st, 128)],
                                     qk[:HEAD_SIZE, :], start=True, stop=True)
                return ps  # scoresT: s_tile st at [:, st*256:(st+1)*256]

            def softmax(sc_ps):
                """P tiles fp16, causal-masked, unnormalized. s_tile 1's left
                half (t < 128) is entirely masked and never computed; its oe
                matmul only covers t >= 128."""
                e0 = soft_pool.tile([128, T], f16, tag="e0")
                nc.scalar.activation(e0[:], sc_ps[:, 0:T], Exp, scale=INV_SQRT_C)
                p0 = soft_pool.tile([128, T], f16, tag="p0")
                nc.vector.tensor_mul(p0[:], e0[:], mask_sb[0][:])
                e1 = soft_pool.tile([128, 128], f16, tag="e1")
                nc.scalar.activation(e1[:], sc_ps[:, 384:512], Exp,
                                     scale=INV_SQRT_C)
                p1 = soft_pool.tile([128, 128], f16, tag="p1")
                nc.vector.tensor_mul(p1[:], e1[:], mask_sb[1][:, 128:T])
                return [p0, p1]

            def oe_mm(v_ext, p_sb):
                ps = ps_pool.tile([128, 512], f32, tag="ps")
                nc.tensor.matmul(ps[:H1, :T], v_ext[0][:], p_sb[0][:],
                                 start=True, stop=False)
                nc.tensor.matmul(ps[:H1, 128:T], v_ext[1][:], p_sb[1][:],
                                 start=False, stop=True, skip_group_check=True)
                sb = out_pool.tile([H1 + 1, T], f16, tag="oe")
                # row 65 is never written by the matmul and never read after the
                # transpose; copying 66 rows keeps partition bases aligned
                nc.vector.tensor_copy(sb[:], ps[:H1 + 1, :T])
                return sb

            def fin_mm(oe):
                ps = ps16_pool.tile([128, 512], f16, tag="ps16")
                for tt in range(2):
                    nc.tensor.transpose(ps[:, tt * 128:tt * 128 + H1 + 1],
                                        oe[:, ts(tt, 128)],
                                        ident[:H1 + 1, :H1 + 1])
                return ps

            def norm_store(b, fin_ps):
                fps = fin_ps[:]
                rz = out_pool.tile([128, 2], f32, tag="rz")
                # both Z columns (offsets 64 and 192) in one strided reciprocal
                nc.vector.reciprocal(rz[:], fps[:, HEAD_SIZE:256:128])
                for tt in range(2):
                    o = out_pool.tile([128, HEAD_SIZE], f32, tag="o")
                    nc.vector.tensor_scalar_mul(
                        o[:], fps[:, tt * 128: tt * 128 + HEAD_SIZE],
                        rz[:, tt:tt + 1])
                    nc.gpsimd.dma_start(out_ap[b, ts(tt, 128), :], o[:])

            # ---- software-pipelined batch loop ----
            # Projections run TWO iterations ahead of their scores matmul so the
            # cross-partition kT DMA has a full iteration of slack.
            # PE stream per iteration:
            #   sc(b) T(b+2) qkT(b+2) v(b+2) oe(b) finT(b-1)
            x_nat = [None] * B_SHARD
            qks, kTs, vs = {}, {}, {}
            x_nat[0] = load_x(0)
            x_nat[1] = load_x(1)
            if B_SHARD > 2:
                x_nat[2] = load_x(2)
            for j in range(min(2, B_SHARD)):
                xtj = transpose_x(x_nat[j])
                qks[j], kTs[j] = qkT_mm(xtj)
                vs[j] = v_mm(xtj, j)
            prev_oe = None
            for b in range(B_SHARD):
                if b + 3 < B_SHARD:
                    x_nat[b + 3] = load_x(b + 3)
                sc_ps = scores_mm(qks.pop(b), kTs.pop(b))
                p_sb = softmax(sc_ps)
                if b + 2 < B_SHARD:
                    xtn = transpose_x(x_nat[b + 2])
                    qks[b + 2], kTs[b + 2] = qkT_mm(xtn)
                    vs[b + 2] = v_mm(xtn, b + 2)
                oe_p = prev_oe
                prev_oe = oe_mm(vs.pop(b), p_sb)
                if oe_p is not None:
                    norm_store(b - 1, fin_mm(oe_p))
            norm_store(B_SHARD - 1, fin_mm(prev_oe))

    nc.compile()
    return nc


def _consts():
    ident = np.eye(128, dtype=np.float16)
    # mask01[st][s_local, t] = 0 where global s > t (causal), else 1
    s = np.arange(T)[:, None]
    t = np.arange(T)[None, :]
    full = (s <= t).astype(np.float16)
    mask01 = np.stack([full[:128], full[128:]], axis=0)
    return ident, mask01


def kernel(x, Wq, Wk, Wv):
    global LAST_RESULTS
    from concourse import bass_utils

    if "nc" not in _CACHE:
        _CACHE["nc"] = _build_program()
    nc = _CACHE["nc"]

    x16 = np.ascontiguousarray(x, dtype=np.float16)
    # [Wq | Wk] stacked on the output dim, chunked along the contraction dim
    wqk = np.concatenate([np.asarray(Wq), np.asarray(Wk)], axis=1)
    wqk16 = np.ascontiguousarray(
        wqk.reshape(CC, 128, 2 * HEAD_SIZE), dtype=np.float16)
    Wv16 = np.ascontiguousarray(Wv, dtype=np.float16)
    ident, mask01 = _consts()

    in_maps = []
    for c in range(N_CORES):
        in_maps.append({
            "x": x16[c * B_SHARD:(c + 1) * B_SHARD],
            "Wqk": wqk16, "Wv": Wv16,
            "ident": ident, "mask01": mask01,
        })

    res = bass_utils.run_bass_kernel_spmd(
        nc, in_maps, core_ids=list(range(N_CORES)), trace=TRACE)
    LAST_RESULTS = res
    out = np.concatenate([res.results[c]["out"] for c in range(N_CORES)], axis=0)
    return np.ascontiguousarray(out, dtype=np.float32)


# revision 22
# speedup vs baseline: 1.0065x; 1.0065x over previous
"""Trainium2 Bass kernel for a causal single-head attention block.

Reference computation (fp32):
    q = x @ Wq; k = x @ Wk; v = x @ Wv        x: [B=256, T=256, C=384], W*: [384, 64]
    wei = softmax(causal_mask(q @ k.T / sqrt(C)))
    out = wei @ v                              out: [256, 256, 64]

Strategy: pure data parallel over B across 8 NeuronCores (32 batches/core).

All matmul operands are fp16: 2-byte operands stream through the PE at full
rate (4-byte fp32/fp32r streams at half rate), get fast weight load, and allow
the DMA engines' 2-byte transpose XBAR. PSUM accumulation stays fp32; measured
end-to-end error vs the fp32 reference is ~8e-4 (scale-relative).

Per-batch dataflow:
    xT    = DMA-transpose(x_b)                 [C, T] fp16 sbuf [128, 768], 3 DMAs,
                                               no PE/DVE involvement at all
    qT|kT = [Wq|Wk].T @ xT                     3 accumulating matmuls, M=128 packed
                                               (q on partitions 0-63, k on 64-127)
    kT    -> own tile via SBUF->SBUF DMA       (cross-partition move, scalar DGE)
    v     = xT.T @ Wv                          [128, 64+64] (both t tiles, one psum)
    sT    = kT[:, s_tile].T @ qT               scores transposed [s, t]
    P     = exp(sT * 1/sqrt(C)) * mask01       exp on ScalarE from psum, 0/1 mask
                                               multiply on DVE (fp16, 2x mode)
    oeT   = [v | 1].T @ P                      [65, 256] psum; row 64 = denominator Z
    out   = PE-transpose(oeT) * (1/Z)          normalize on ScalarE, DMA out (GpSimd
                                               DGE so the sync queue stays short)

Software pipeline (PE stream per iteration):
    sc(b) qkT(b+1) v(b+1) oe(b) finT(b-1)
so the softmax chain of batch b (ACT exp + DVE mask) runs under the next
batch's projections, and finT's oe-copy dependency is a full iteration old.
"""

import numpy as np

N_EMBED = 384
HEAD_SIZE = 64
H1 = HEAD_SIZE + 1
T = 256
B = 256
N_CORES = 8
B_SHARD = B // N_CORES  # 32
CC = N_EMBED // 128  # 3 contraction chunks
INV_SQRT_C = 1.0 / float(np.sqrt(N_EMBED))

_CACHE = {}

# test.py can flip these before calling kernel()
TRACE = False
LAST_RESULTS = None


def _build_program():
    import concourse.bacc as bacc
    import concourse.mybir as mybir
    import concourse.tile as tile
    from concourse import bass

    f32 = mybir.dt.float32
    f16 = mybir.dt.float16
    ts = bass.ts
    Exp = mybir.ActivationFunctionType.Exp
    Copy = mybir.ActivationFunctionType.Copy

    nc = bacc.Bacc("TRN2", target_bir_lowering=False, debug=False,
                   enable_asserts=False)

    x_d = nc.dram_tensor("x", [B_SHARD, T, N_EMBED], f16, kind="ExternalInput")
    wqk_d = nc.dram_tensor("Wqk", [CC, 128, 128], f16, kind="ExternalInput")
    wv_d = nc.dram_tensor("Wv", [N_EMBED, HEAD_SIZE], f16, kind="ExternalInput")
    ident_d = nc.dram_tensor("ident", [128, 128], f16, kind="ExternalInput")
    mask_d = nc.dram_tensor("mask01", [2, 128, T], f16, kind="ExternalInput")
    out_d = nc.dram_tensor("out", [B_SHARD, T, HEAD_SIZE], f32, kind="ExternalOutput")

    x_ap = x_d.ap()
    out_ap = out_d.ap()

    with tile.TileContext(nc) as tc:
        with (
            tc.tile_pool(name="const", bufs=1) as cpool,
            tc.tile_pool(name="xin", bufs=6) as xin_pool,
            tc.tile_pool(name="xt", bufs=2) as xt_pool,
            tc.tile_pool(name="proj", bufs=7) as proj_pool,
            tc.tile_pool(name="soft", bufs=6) as soft_pool,
            tc.tile_pool(name="outp", bufs=8) as out_pool,
            tc.tile_pool(name="ps", bufs=5, space="PSUM") as ps_pool,
            tc.tile_pool(name="ps16", bufs=3, space="PSUM") as ps16_pool,
        ):
            # ---- constants ----
            ident = cpool.tile([128, 128], f16, tag="ident")
            nc.sync.dma_start(ident[:], ident_d.ap())
            wqk_sb, wv_sb = [], []
            for cc in range(CC):
                t_ = cpool.tile([128, 128], f16, tag=f"wqk{cc}")
                nc.sync.dma_start(t_[:], wqk_d.ap()[cc])
                wqk_sb.append(t_)
                t_ = cpool.tile([128, HEAD_SIZE], f16, tag=f"wv{cc}")
                nc.sync.dma_start(t_[:], wv_d.ap()[ts(cc, 128), :])
                wv_sb.append(t_)
            mask_sb = []
            for st in range(2):
                t_ = cpool.tile([128, T], f16, tag=f"mask{st}")
                nc.sync.dma_start(t_[:], mask_d.ap()[st, :, :])
                mask_sb.append(t_)
            ones_col = cpool.tile([128, 1], f16, tag="ones")
            nc.gpsimd.memset(ones_col[:], 1.0)

            def load_x(b):
                tiles = []
                for tt in range(2):
                    t_ = xin_pool.tile([128, N_EMBED], f16, tag="x_nat")
                    nc.sync.dma_start(t_[:], x_ap[b, ts(tt, 128), :])
                    tiles.append(t_)
                return tiles

            def transpose_x(x_nat):
                """-> one [128, 768] fp16 sbuf tile; chunk cc at [:, cc*256:(cc+1)*256]"""
                ps = ps16_pool.tile([128, 3 * T], f16, tag="ps16")
                for cc in range(CC):
                    for tt in range(2):
                        nc.tensor.transpose(
                            ps[:, cc * T + tt * 128: cc * T + (tt + 1) * 128],
                            x_nat[tt][:, ts(cc, 128)], ident[:])
                sb = xt_pool.tile([128, 3 * T], f16, tag="xt")
                nc.vector.tensor_copy(sb[:], ps[:])
                return sb

            def qkT_mm(xt):
                """q and k projections in one M=128 stack; kT moved to its own
                base-0 tile with a cross-partition SBUF->SBUF DMA."""
                ps = ps_pool.tile([128, 512], f32, tag="ps")
                for cc in range(CC):
                    nc.tensor.matmul(ps[:, :T], wqk_sb[cc][:], xt[:, ts(cc, T)],
                                     start=(cc == 0), stop=(cc == CC - 1))
                sb = proj_pool.tile([128, T], f16, tag="qk")
                nc.vector.tensor_copy(sb[:], ps[:, :T])
                kT = proj_pool.tile([HEAD_SIZE, T], f16, tag="kT")
                nc.sync.dma_start(kT[:], sb[HEAD_SIZE:128, :])
                return sb, kT  # qT = sb[:64]

            def v_mm(xt):
                ps = ps_pool.tile([128, 512], f32, tag="ps")
                for tt in range(2):
                    for cc in range(CC):
                        nc.tensor.matmul(ps[:, tt * 128:tt * 128 + HEAD_SIZE],
                                         xt[:, cc * T + tt * 128: cc * T + (tt + 1) * 128],
                                         wv_sb[cc][:],
                                         start=(cc == 0), stop=(cc == CC - 1))
                v_ext = []
                for tt in range(2):
                    sb = proj_pool.tile([128, H1], f16, tag="v_ext")
                    nc.vector.tensor_copy(sb[:, :HEAD_SIZE],
                                          ps[:, tt * 128:tt * 128 + HEAD_SIZE])
                    nc.vector.tensor_copy(sb[:, HEAD_SIZE:H1], ones_col[:])
                    v_ext.append(sb)
                return v_ext

            def scores_mm(qk, kT):
                ps = ps_pool.tile([128, 512], f32, tag="ps")
                for st in range(2):
                    nc.tensor.matmul(ps[:, ts(st, T)], kT[:, ts(st, 128)],
                                     qk[:HEAD_SIZE, :], start=True, stop=True)
                return ps  # scoresT: s_tile st at [:, st*256:(st+1)*256]

            def softmax(sc_ps):
                """P [128, 256] fp16 per s_tile: exp then 0/1-mask, unnormalized."""
                p_sb = []
                for st in range(2):
                    e = soft_pool.tile([128, T], f16, tag=f"e{st}")
                    nc.scalar.activation(e[:], sc_ps[:, ts(st, T)], Exp,
                                         scale=INV_SQRT_C)
                    p = soft_pool.tile([128, T], f16, tag=f"p{st}")
                    nc.vector.tensor_mul(p[:], e[:], mask_sb[st][:])
                    p_sb.append(p)
                return p_sb

            def oe_mm(v_ext, p_sb):
                ps = ps_pool.tile([128, 512], f32, tag="ps")
                for st in range(2):
                    nc.tensor.matmul(ps[:H1, :T], v_ext[st][:], p_sb[st][:],
                                     start=(st == 0), stop=(st == 1))
                sb = out_pool.tile([H1 + 1, T], f16, tag="oe")
                # row 65 is never written by the matmul and never read after the
                # transpose; copying 66 rows keeps partition bases aligned
                nc.vector.tensor_copy(sb[:], ps[:H1 + 1, :T])
                return sb

            def fin_mm(oe):
                ps = ps16_pool.tile([128, 512], f16, tag="ps16")
                for tt in range(2):
                    nc.tensor.transpose(ps[:, tt * 128:tt * 128 + H1 + 1],
                                        oe[:, ts(tt, 128)],
                                        ident[:H1 + 1, :H1 + 1])
                return ps

            def norm_store(b, fin_ps):
                fps = fin_ps[:]
                rz = out_pool.tile([128, 2], f32, tag="rz")
                # both Z columns (offsets 64 and 192) in one strided reciprocal
                nc.vector.reciprocal(rz[:], fps[:, HEAD_SIZE:256:128])
                for tt in range(2):
                    o = out_pool.tile([128, HEAD_SIZE], f32, tag="o")
                    nc.vector.tensor_scalar_mul(
                        o[:], fps[:, tt * 128: tt * 128 + HEAD_SIZE],
                        rz[:, tt:tt + 1])
                    nc.gpsimd.dma_start(out_ap[b, ts(tt, 128), :], o[:])

            # ---- software-pipelined batch loop ----
            # Projections run TWO iterations ahead of their scores matmul so the
            # cross-partition kT DMA has a full iteration of slack.
            # PE stream per iteration:
            #   sc(b) T(b+2) qkT(b+2) v(b+2) oe(b) finT(b-1)
            x_nat = [None] * B_SHARD
            qks, kTs, vs = {}, {}, {}
            x_nat[0] = load_x(0)
            x_nat[1] = load_x(1)
            if B_SHARD > 2:
                x_nat[2] = load_x(2)
            for j in range(min(2, B_SHARD)):
                xtj = transpose_x(x_nat[j])
                qks[j], kTs[j] = qkT_mm(xtj)
                vs[j] = v_mm(xtj)
            prev_oe = None
            for b in range(B_SHARD):
                if b + 3 < B_SHARD:
                    x_nat[b + 3] = load_x(b + 3)
                sc_ps = scores_mm(qks.pop(b), kTs.pop(b))
                p_sb = softmax(sc_ps)
                if b + 2 < B_SHARD:
                    xtn = transpose_x(x_nat[b + 2])
                    qks[b + 2], kTs[b + 2] = qkT_mm(xtn)
                    vs[b + 2] = v_mm(xtn)
                oe_p = prev_oe
                prev_oe = oe_mm(vs.pop(b), p_sb)
                if oe_p is not None:
                    norm_store(b - 1, fin_mm(oe_p))
            norm_store(B_SHARD - 1, fin_mm(prev_oe))

    nc.compile()
    return nc


def _consts():
    ident = np.eye(128, dtype=np.float16)
    # mask01[st][s_local, t] = 0 where global s > t (causal), else 1
    s = np.arange(T)[:, None]
    t = np.arange(T)[None, :]
    full = (s <= t).astype(np.float16)
    mask01 = np.stack([full[:128], full[128:]], axis=0)
    return ident, mask01


def kernel(x, Wq, Wk, Wv):
    global LAST_RESULTS
    from concourse import bass_utils

    if "nc" not in _CACHE:
        _CACHE["nc"] = _build_program()
    nc = _CACHE["nc"]

    x16 = np.ascontiguousarray(x, dtype=np.float16)
    # [Wq | Wk] stacked on the output dim, chunked along the contraction dim
    wqk = np.concatenate([np.asarray(Wq), np.asarray(Wk)], axis=1)
    wqk16 = np.ascontiguousarray(
        wqk.reshape(CC, 128, 2 * HEAD_SIZE), dtype=np.float16)
    Wv16 = np.ascontiguousarray(Wv, dtype=np.float16)
    ident, mask01 = _consts()

    in_maps = []
    for c in range(N_CORES):
        in_maps.append({
            "x": x16[c * B_SHARD:(c + 1) * B_SHARD],
            "Wqk": wqk16, "Wv": Wv16,
            "ident": ident, "mask01": mask01,
        })

    res = bass_utils.run_bass_kernel_spmd(
        nc, in_maps, core_ids=list(range(N_CORES)), trace=TRACE)
    LAST_RESULTS = res
    out = np.concatenate([res.results[c]["out"] for c in range(N_CORES)], axis=0)
    return np.ascontiguousarray(out, dtype=np.float32)


# revision 23
# speedup vs baseline: 1.0314x; 1.0248x over previous
"""Trainium2 Bass kernel for a causal single-head attention block.

Reference computation (fp32):
    q = x @ Wq; k = x @ Wk; v = x @ Wv        x: [B=256, T=256, C=384], W*: [384, 64]
    wei = softmax(causal_mask(q @ k.T / sqrt(C)))
    out = wei @ v                              out: [256, 256, 64]

Strategy: pure data parallel over B across 8 NeuronCores (32 batches/core).

All matmul operands are fp16: 2-byte operands stream through the PE at full
rate (4-byte fp32/fp32r streams at half rate), get fast weight load, and allow
the DMA engines' 2-byte transpose XBAR. PSUM accumulation stays fp32; measured
end-to-end error vs the fp32 reference is ~8e-4 (scale-relative).

Per-batch dataflow:
    xT    = DMA-transpose(x_b)                 [C, T] fp16 sbuf [128, 768], 3 DMAs,
                                               no PE/DVE involvement at all
    qT|kT = [Wq|Wk].T @ xT                     3 accumulating matmuls, M=128 packed
                                               (q on partitions 0-63, k on 64-127)
    kT    -> own tile via SBUF->SBUF DMA       (cross-partition move, scalar DGE)
    v     = xT.T @ Wv                          [128, 64+64] (both t tiles, one psum)
    sT    = kT[:, s_tile].T @ qT               scores transposed [s, t]
    P     = exp(sT * 1/sqrt(C)) * mask01       exp on ScalarE from psum, 0/1 mask
                                               multiply on DVE (fp16, 2x mode)
    oeT   = [v | 1].T @ P                      [65, 256] psum; row 64 = denominator Z
    out   = PE-transpose(oeT) * (1/Z)          normalize on ScalarE, DMA out (GpSimd
                                               DGE so the sync queue stays short)

Software pipeline (PE stream per iteration):
    sc(b) qkT(b+1) v(b+1) oe(b) finT(b-1)
so the softmax chain of batch b (ACT exp + DVE mask) runs under the next
batch's projections, and finT's oe-copy dependency is a full iteration old.
"""

import numpy as np

N_EMBED = 384
HEAD_SIZE = 64
H1 = HEAD_SIZE + 1
T = 256
B = 256
N_CORES = 8
B_SHARD = B // N_CORES  # 32
CC = N_EMBED // 128  # 3 contraction chunks
INV_SQRT_C = 1.0 / float(np.sqrt(N_EMBED))

_CACHE = {}

# test.py can flip these before calling kernel()
TRACE = False
LAST_RESULTS = None


def _build_program():
    import concourse.bacc as bacc
    import concourse.mybir as mybir
    import concourse.tile as tile
    from concourse import bass

    f32 = mybir.dt.float32
    f16 = mybir.dt.float16
    ts = bass.ts
    Exp = mybir.ActivationFunctionType.Exp
    Copy = mybir.ActivationFunctionType.Copy

    nc = bacc.Bacc("TRN2", target_bir_lowering=False, debug=False,
                   enable_asserts=False)

    x_d = nc.dram_tensor("x", [B_SHARD, T, N_EMBED], f16, kind="ExternalInput")
    wqk_d = nc.dram_tensor("Wqk", [CC, 128, 128], f16, kind="ExternalInput")
    wv_d = nc.dram_tensor("Wv", [N_EMBED, HEAD_SIZE], f16, kind="ExternalInput")
    ident_d = nc.dram_tensor("ident", [128, 128], f16, kind="ExternalInput")
    mask_d = nc.dram_tensor("mask01", [2, 128, T], f16, kind="ExternalInput")
    out_d = nc.dram_tensor("out", [B_SHARD, T, HEAD_SIZE], f32, kind="ExternalOutput")

    x_ap = x_d.ap()
    out_ap = out_d.ap()

    with tile.TileContext(nc) as tc:
        with (
            tc.tile_pool(name="const", bufs=1) as cpool,
            tc.tile_pool(name="xin", bufs=6) as xin_pool,
            tc.tile_pool(name="xt", bufs=2) as xt_pool,
            tc.tile_pool(name="proj", bufs=7) as proj_pool,
            tc.tile_pool(name="soft", bufs=6) as soft_pool,
            tc.tile_pool(name="outp", bufs=8) as out_pool,
            tc.tile_pool(name="ps", bufs=5, space="PSUM") as ps_pool,
            tc.tile_pool(name="ps16", bufs=3, space="PSUM") as ps16_pool,
        ):
            # ---- constants ----
            ident = cpool.tile([128, 128], f16, tag="ident")
            nc.sync.dma_start(ident[:], ident_d.ap())
            wqk_sb, wv_sb = [], []
            for cc in range(CC):
                t_ = cpool.tile([128, 128], f16, tag=f"wqk{cc}")
                nc.sync.dma_start(t_[:], wqk_d.ap()[cc])
                wqk_sb.append(t_)
                t_ = cpool.tile([128, HEAD_SIZE], f16, tag=f"wv{cc}")
                nc.sync.dma_start(t_[:], wv_d.ap()[ts(cc, 128), :])
                wv_sb.append(t_)
            mask_sb = []
            for st in range(2):
                t_ = cpool.tile([128, T], f16, tag=f"mask{st}")
                nc.sync.dma_start(t_[:], mask_d.ap()[st, :, :])
                mask_sb.append(t_)
            ones_col = cpool.tile([128, 1], f16, tag="ones")
            nc.gpsimd.memset(ones_col[:], 1.0)

            def load_x(b):
                tiles = []
                for tt in range(2):
                    t_ = xin_pool.tile([128, N_EMBED], f16, tag="x_nat")
                    nc.sync.dma_start(t_[:], x_ap[b, ts(tt, 128), :])
                    tiles.append(t_)
                return tiles

            def transpose_x(x_nat):
                """-> one [128, 768] fp16 sbuf tile; chunk cc at [:, cc*256:(cc+1)*256]"""
                ps = ps16_pool.tile([128, 3 * T], f16, tag="ps16")
                for cc in range(CC):
                    for tt in range(2):
                        nc.tensor.transpose(
                            ps[:, cc * T + tt * 128: cc * T + (tt + 1) * 128],
                            x_nat[tt][:, ts(cc, 128)], ident[:])
                sb = xt_pool.tile([128, 3 * T], f16, tag="xt")
                nc.vector.tensor_copy(sb[:], ps[:])
                return sb

            def qkT_mm(xt):
                """q and k projections in one M=128 stack; kT moved to its own
                base-0 tile with a cross-partition SBUF->SBUF DMA."""
                ps = ps_pool.tile([128, 512], f32, tag="ps")
                for cc in range(CC):
                    nc.tensor.matmul(ps[:, :T], wqk_sb[cc][:], xt[:, ts(cc, T)],
                                     start=(cc == 0), stop=(cc == CC - 1))
                sb = proj_pool.tile([128, T], f16, tag="qk")
                nc.vector.tensor_copy(sb[:], ps[:, :T])
                kT = proj_pool.tile([HEAD_SIZE, T], f16, tag="kT")
                nc.sync.dma_start(kT[:], sb[HEAD_SIZE:128, :])
                return sb, kT  # qT = sb[:64]

            def v_mm(xt):
                ps = ps_pool.tile([128, 512], f32, tag="ps")
                for tt in range(2):
                    for cc in range(CC):
                        nc.tensor.matmul(ps[:, tt * 128:tt * 128 + HEAD_SIZE],
                                         xt[:, cc * T + tt * 128: cc * T + (tt + 1) * 128],
                                         wv_sb[cc][:],
                                         start=(cc == 0), stop=(cc == CC - 1))
                v_ext = []
                for tt in range(2):
                    sb = proj_pool.tile([128, H1], f16, tag="v_ext")
                    nc.vector.tensor_copy(sb[:, :HEAD_SIZE],
                                          ps[:, tt * 128:tt * 128 + HEAD_SIZE])
                    nc.vector.tensor_copy(sb[:, HEAD_SIZE:H1], ones_col[:])
                    v_ext.append(sb)
                return v_ext

            def scores_mm(qk, kT):
                ps = ps_pool.tile([128, 512], f32, tag="ps")
                for st in range(2):
                    nc.tensor.matmul(ps[:, ts(st, T)], kT[:, ts(st, 128)],
                                     qk[:HEAD_SIZE, :], start=True, stop=True)
                return ps  # scoresT: s_tile st at [:, st*256:(st+1)*256]

            def softmax(sc_ps):
                """P tiles fp16, causal-masked, unnormalized. s_tile 1's left
                half (t < 128) is entirely masked and never computed; its oe
                matmul only covers t >= 128."""
                e0 = soft_pool.tile([128, T], f16, tag="e0")
                nc.scalar.activation(e0[:], sc_ps[:, 0:T], Exp, scale=INV_SQRT_C)
                p0 = soft_pool.tile([128, T], f16, tag="p0")
                nc.vector.tensor_mul(p0[:], e0[:], mask_sb[0][:])
                e1 = soft_pool.tile([128, 128], f16, tag="e1")
                nc.scalar.activation(e1[:], sc_ps[:, 384:512], Exp,
                                     scale=INV_SQRT_C)
                p1 = soft_pool.tile([128, 128], f16, tag="p1")
                nc.vector.tensor_mul(p1[:], e1[:], mask_sb[1][:, 128:T])
                return [p0, p1]

            def oe_mm(v_ext, p_sb):
                ps = ps_pool.tile([128, 512], f32, tag="ps")
                nc.tensor.matmul(ps[:H1, :T], v_ext[0][:], p_sb[0][:],
                                 start=True, stop=False)
                nc.tensor.matmul(ps[:H1, 128:T], v_ext[1][:], p_sb[1][:],
                                 start=False, stop=True, skip_group_check=True)
                sb = out_pool.tile([H1 + 1, T], f16, tag="oe")
                # row 65 is never written by the matmul and never read after the
                # transpose; copying 66 rows keeps partition bases aligned
                nc.vector.tensor_copy(sb[:], ps[:H1 + 1, :T])
                return sb

            def fin_mm(oe):
                ps = ps16_pool.tile([128, 512], f16, tag="ps16")
                for tt in range(2):
                    nc.tensor.transpose(ps[:, tt * 128:tt * 128 + H1 + 1],
                                        oe[:, ts(tt, 128)],
                                        ident[:H1 + 1, :H1 + 1])
                return ps

            def norm_store(b, fin_ps):
                fps = fin_ps[:]
                rz = out_pool.tile([128, 2], f32, tag="rz")
                # both Z columns (offsets 64 and 192) in one strided reciprocal
                nc.vector.reciprocal(rz[:], fps[:, HEAD_SIZE:256:128])
                for tt in range(2):
                    o = out_pool.tile([128, HEAD_SIZE], f32, tag="o")
                    nc.scalar.activation(o[:], fps[:, tt * 128: tt * 128 + HEAD_SIZE],
                                         Copy, scale=rz[:, tt:tt + 1])
                    nc.gpsimd.dma_start(out_ap[b, ts(tt, 128), :], o[:])

            # ---- software-pipelined batch loop ----
            # Projections run TWO iterations ahead of their scores matmul so the
            # cross-partition kT DMA has a full iteration of slack.
            # PE stream per iteration:
            #   sc(b) T(b+2) qkT(b+2) v(b+2) oe(b) finT(b-1)
            x_nat = [None] * B_SHARD
            qks, kTs, vs = {}, {}, {}
            x_nat[0] = load_x(0)
            x_nat[1] = load_x(1)
            if B_SHARD > 2:
                x_nat[2] = load_x(2)
            for j in range(min(2, B_SHARD)):
                xtj = transpose_x(x_nat[j])
                qks[j], kTs[j] = qkT_mm(xtj)
                vs[j] = v_mm(xtj)
            prev_oe = None
            for b in range(B_SHARD):
                if b + 3 < B_SHARD:
                    x_nat[b + 3] = load_x(b + 3)
                sc_ps = scores_mm(qks.pop(b), kTs.pop(b))
                p_sb = softmax(sc_ps)
                if b + 2 < B_SHARD:
                    xtn = transpose_x(x_nat[b + 2])
                    qks[b + 2], kTs[b + 2] = qkT_mm(xtn)
                    vs[b + 2] = v_mm(xtn)
                oe_p = prev_oe
                prev_oe = oe_mm(vs.pop(b), p_sb)
                if oe_p is not None:
                    norm_store(b - 1, fin_mm(oe_p))
            norm_store(B_SHARD - 1, fin_mm(prev_oe))

    nc.compile()
    return nc


def _consts():
    ident = np.eye(128, dtype=np.float16)
    # mask01[st][s_local, t] = 0 where global s > t (causal), else 1
    s = np.arange(T)[:, None]
    t = np.arange(T)[None, :]
    full = (s <= t).astype(np.float16)
    mask01 = np.stack([full[:128], full[128:]], axis=0)
    return ident, mask01


def kernel(x, Wq, Wk, Wv):
    global LAST_RESULTS
    from concourse import bass_utils

    if "nc" not in _CACHE:
        _CACHE["nc"] = _build_program()
    nc = _CACHE["nc"]

    x16 = np.ascontiguousarray(x, dtype=np.float16)
    # [Wq | Wk] stacked on the output dim, chunked along the contraction dim
    wqk = np.concatenate([np.asarray(Wq), np.asarray(Wk)], axis=1)
    wqk16 = np.ascontiguousarray(
        wqk.reshape(CC, 128, 2 * HEAD_SIZE), dtype=np.float16)
    Wv16 = np.ascontiguousarray(Wv, dtype=np.float16)
    ident, mask01 = _consts()

    in_maps = []
    for c in range(N_CORES):
        in_maps.append({
            "x": x16[c * B_SHARD:(c + 1) * B_SHARD],
            "Wqk": wqk16, "Wv": Wv16,
            "ident": ident, "mask01": mask01,
        })

    res = bass_utils.run_bass_kernel_spmd(
        nc, in_maps, core_ids=list(range(N_CORES)), trace=TRACE)
    LAST_RESULTS = res
    out = np.concatenate([res.results[c]["out"] for c in range(N_CORES)], axis=0)
    return np.ascontiguousarray(out, dtype=np.float32)


# revision 24
# speedup vs baseline: 1.1062x; 1.0725x over previous
"""Trainium2 Bass kernel for a causal single-head attention block.

Reference computation (fp32):
    q = x @ Wq; k = x @ Wk; v = x @ Wv        x: [B=256, T=256, C=384], W*: [384, 64]
    wei = softmax(causal_mask(q @ k.T / sqrt(C)))
    out = wei @ v                              out: [256, 256, 64]

Strategy: pure data parallel over B across 8 NeuronCores (32 batches/core).

All matmul operands are fp16: 2-byte operands stream through the PE at full
rate (4-byte fp32/fp32r streams at half rate), get fast weight load, and allow
the DMA engines' 2-byte transpose XBAR. PSUM accumulation stays fp32; measured
end-to-end error vs the fp32 reference is ~8e-4 (scale-relative).

Per-batch dataflow:
    xT    = DMA-transpose(x_b)                 [C, T] fp16 sbuf [128, 768], 3 DMAs,
                                               no PE/DVE involvement at all
    qT|kT = [Wq|Wk].T @ xT                     3 accumulating matmuls, M=128 packed
                                               (q on partitions 0-63, k on 64-127)
    kT    -> own tile via SBUF->SBUF DMA       (cross-partition move, scalar DGE)
    v     = xT.T @ Wv                          [128, 64+64] (both t tiles, one psum)
    sT    = kT[:, s_tile].T @ qT               scores transposed [s, t]
    P     = exp(sT * 1/sqrt(C)) * mask01       exp on ScalarE from psum, 0/1 mask
                                               multiply on DVE (fp16, 2x mode)
    oeT   = [v | 1].T @ P                      [65, 256] psum; row 64 = denominator Z
    out   = PE-transpose(oeT) * (1/Z)          normalize on ScalarE, DMA out (GpSimd
                                               DGE so the sync queue stays short)

Software pipeline (PE stream per iteration):
    sc(b) qkT(b+1) v(b+1) oe(b) finT(b-1)
so the softmax chain of batch b (ACT exp + DVE mask) runs under the next
batch's projections, and finT's oe-copy dependency is a full iteration old.
"""

import numpy as np

N_EMBED = 384
HEAD_SIZE = 64
H1 = HEAD_SIZE + 1
T = 256
B = 256
N_CORES = 8
B_SHARD = B // N_CORES  # 32
CC = N_EMBED // 128  # 3 contraction chunks
INV_SQRT_C = 1.0 / float(np.sqrt(N_EMBED))

_CACHE = {}

# test.py can flip these before calling kernel()
TRACE = False
LAST_RESULTS = None


def _build_program():
    import concourse.bacc as bacc
    import concourse.mybir as mybir
    import concourse.tile as tile
    from concourse import bass

    f32 = mybir.dt.float32
    f16 = mybir.dt.float16
    ts = bass.ts
    Exp = mybir.ActivationFunctionType.Exp
    Copy = mybir.ActivationFunctionType.Copy

    nc = bacc.Bacc("TRN2", target_bir_lowering=False, debug=False,
                   enable_asserts=False)

    x_d = nc.dram_tensor("x", [B_SHARD, T, N_EMBED], f16, kind="ExternalInput")
    wqk_d = nc.dram_tensor("Wqk", [CC, 128, 128], f16, kind="ExternalInput")
    wv_d = nc.dram_tensor("Wv", [N_EMBED, HEAD_SIZE], f16, kind="ExternalInput")
    ident_d = nc.dram_tensor("ident", [128, 128], f16, kind="ExternalInput")
    mask_d = nc.dram_tensor("mask01", [2, 128, T], f16, kind="ExternalInput")
    out_d = nc.dram_tensor("out", [B_SHARD, T, HEAD_SIZE], f32, kind="ExternalOutput")

    x_ap = x_d.ap()
    out_ap = out_d.ap()

    with tile.TileContext(nc) as tc:
        with (
            tc.tile_pool(name="const", bufs=1) as cpool,
            tc.tile_pool(name="xin", bufs=6) as xin_pool,
            tc.tile_pool(name="xt", bufs=2) as xt_pool,
            tc.tile_pool(name="proj", bufs=7) as proj_pool,
            tc.tile_pool(name="soft", bufs=6) as soft_pool,
            tc.tile_pool(name="outp", bufs=8) as out_pool,
            tc.tile_pool(name="ps", bufs=5, space="PSUM") as ps_pool,
            tc.tile_pool(name="ps16", bufs=3, space="PSUM") as ps16_pool,
        ):
            # ---- constants ----
            ident = cpool.tile([128, 128], f16, tag="ident")
            nc.sync.dma_start(ident[:], ident_d.ap())
            wqk_sb, wv_sb = [], []
            for cc in range(CC):
                t_ = cpool.tile([128, 128], f16, tag=f"wqk{cc}")
                nc.sync.dma_start(t_[:], wqk_d.ap()[cc])
                wqk_sb.append(t_)
                t_ = cpool.tile([128, HEAD_SIZE], f16, tag=f"wv{cc}")
                nc.sync.dma_start(t_[:], wv_d.ap()[ts(cc, 128), :])
                wv_sb.append(t_)
            mask_sb = []
            for st in range(2):
                t_ = cpool.tile([128, T], f16, tag=f"mask{st}")
                nc.sync.dma_start(t_[:], mask_d.ap()[st, :, :])
                mask_sb.append(t_)
            ones_col = cpool.tile([128, 1], f16, tag="ones")
            nc.gpsimd.memset(ones_col[:], 1.0)

            def load_x(b):
                tiles = []
                for tt in range(2):
                    t_ = xin_pool.tile([128, N_EMBED], f16, tag="x_nat")
                    nc.sync.dma_start(t_[:], x_ap[b, ts(tt, 128), :])
                    tiles.append(t_)
                return tiles

            def transpose_x(x_nat):
                """-> one [128, 768] fp16 sbuf tile; chunk cc at [:, cc*256:(cc+1)*256]"""
                ps = ps16_pool.tile([128, 3 * T], f16, tag="ps16")
                for cc in range(CC):
                    for tt in range(2):
                        nc.tensor.transpose(
                            ps[:, cc * T + tt * 128: cc * T + (tt + 1) * 128],
                            x_nat[tt][:, ts(cc, 128)], ident[:])
                sb = xt_pool.tile([128, 3 * T], f16, tag="xt")
                nc.vector.tensor_copy(sb[:], ps[:])
                return sb

            def qkT_mm(xt):
                """q and k projections in one M=128 stack; kT moved to its own
                base-0 tile with a cross-partition SBUF->SBUF DMA."""
                ps = ps_pool.tile([128, 512], f32, tag="ps")
                for cc in range(CC):
                    nc.tensor.matmul(ps[:, :T], wqk_sb[cc][:], xt[:, ts(cc, T)],
                                     start=(cc == 0), stop=(cc == CC - 1))
                sb = proj_pool.tile([128, T], f16, tag="qk")
                nc.vector.tensor_copy(sb[:], ps[:, :T])
                kT = proj_pool.tile([HEAD_SIZE, T], f16, tag="kT")
                nc.sync.dma_start(kT[:], sb[HEAD_SIZE:128, :])
                return sb, kT  # qT = sb[:64]

            def v_mm(xt):
                ps = ps_pool.tile([128, 512], f32, tag="ps")
                for tt in range(2):
                    for cc in range(CC):
                        nc.tensor.matmul(ps[:, tt * 128:tt * 128 + HEAD_SIZE],
                                         xt[:, cc * T + tt * 128: cc * T + (tt + 1) * 128],
                                         wv_sb[cc][:],
                                         start=(cc == 0), stop=(cc == CC - 1))
                v_ext = []
                for tt in range(2):
                    sb = proj_pool.tile([128, H1], f16, tag="v_ext")
                    nc.vector.tensor_copy(sb[:, :HEAD_SIZE],
                                          ps[:, tt * 128:tt * 128 + HEAD_SIZE])
                    nc.vector.tensor_copy(sb[:, HEAD_SIZE:H1], ones_col[:])
                    v_ext.append(sb)
                return v_ext

            def scores_mm(qk, kT):
                ps = ps_pool.tile([128, 512], f32, tag="ps")
                for st in range(2):
                    nc.tensor.matmul(ps[:, ts(st, T)], kT[:, ts(st, 128)],
                                     qk[:HEAD_SIZE, :], start=True, stop=True)
                return ps  # scoresT: s_tile st at [:, st*256:(st+1)*256]

            def softmax(sc_ps):
                """P [128, 256] fp16 per s_tile: exp then 0/1-mask, unnormalized."""
                p_sb = []
                for st in range(2):
                    e = soft_pool.tile([128, T], f16, tag=f"e{st}")
                    nc.scalar.activation(e[:], sc_ps[:, ts(st, T)], Exp,
                                         scale=INV_SQRT_C)
                    p = soft_pool.tile([128, T], f16, tag=f"p{st}")
                    nc.vector.tensor_mul(p[:], e[:], mask_sb[st][:])
                    p_sb.append(p)
                return p_sb

            def oe_mm(v_ext, p_sb):
                ps = ps_pool.tile([128, 512], f32, tag="ps")
                for st in range(2):
                    nc.tensor.matmul(ps[:H1, :T], v_ext[st][:], p_sb[st][:],
                                     start=(st == 0), stop=(st == 1))
                sb = out_pool.tile([H1 + 1, T], f16, tag="oe")
                # row 65 is never written by the matmul and never read after the
                # transpose; copying 66 rows keeps partition bases aligned
                nc.vector.tensor_copy(sb[:], ps[:H1 + 1, :T])
                return sb

            def fin_mm(oe):
                ps = ps16_pool.tile([128, 512], f16, tag="ps16")
                for tt in range(2):
                    nc.tensor.transpose(ps[:, tt * 128:tt * 128 + H1 + 1],
                                        oe[:, ts(tt, 128)],
                                        ident[:H1 + 1, :H1 + 1])
                return ps

            def norm_store(b, fin_ps):
                fps = fin_ps[:]
                rz = out_pool.tile([128, 2], f32, tag="rz")
                # both Z columns (offsets 64 and 192) in one strided reciprocal
                nc.vector.reciprocal(rz[:], fps[:, HEAD_SIZE:256:128])
                for tt in range(2):
                    o = out_pool.tile([128, HEAD_SIZE], f32, tag="o")
                    nc.scalar.activation(o[:], fps[:, tt * 128: tt * 128 + HEAD_SIZE],
                                         Copy, scale=rz[:, tt:tt + 1])
                    nc.gpsimd.dma_start(out_ap[b, ts(tt, 128), :], o[:])

            # ---- software-pipelined batch loop ----
            # Projections run TWO iterations ahead of their scores matmul so the
            # cross-partition kT DMA has a full iteration of slack.
            # PE stream per iteration:
            #   sc(b) T(b+2) qkT(b+2) v(b+2) oe(b) finT(b-1)
            x_nat = [None] * B_SHARD
            qks, kTs, vs = {}, {}, {}
            x_nat[0] = load_x(0)
            x_nat[1] = load_x(1)
            if B_SHARD > 2:
                x_nat[2] = load_x(2)
            for j in range(min(2, B_SHARD)):
                xtj = transpose_x(x_nat[j])
                qks[j], kTs[j] = qkT_mm(xtj)
                vs[j] = v_mm(xtj)
            prev_oe = None
            for b in range(B_SHARD):
                if b + 3 < B_SHARD:
                    x_nat[b + 3] = load_x(b + 3)
                sc_ps = scores_mm(qks.pop(b), kTs.pop(b))
                p_sb = softmax(sc_ps)
                if b + 2 < B_SHARD:
                    xtn = transpose_x(x_nat[b + 2])
                    qks[b + 2], kTs[b + 2] = qkT_mm(xtn)
                    vs[b + 2] = v_mm(xtn)
                oe_p = prev_oe
                prev_oe = oe_mm(vs.pop(b), p_sb)
                if oe_p is not None:
                    norm_store(b - 1, fin_mm(oe_p))
            norm_store(B_SHARD - 1, fin_mm(prev_oe))

    nc.compile()
    return nc


def _consts():
    ident = np.eye(128, dtype=np.float16)
    # mask01[st][s_local, t] = 0 where global s > t (causal), else 1
    s = np.arange(T)[:, None]
    t = np.arange(T)[None, :]
    full = (s <= t).astype(np.float16)
    mask01 = np.stack([full[:128], full[128:]], axis=0)
    return ident, mask01


def kernel(x, Wq, Wk, Wv):
    global LAST_RESULTS
    from concourse import bass_utils

    if "nc" not in _CACHE:
        _CACHE["nc"] = _build_program()
    nc = _CACHE["nc"]

    x16 = np.ascontiguousarray(x, dtype=np.float16)
    # [Wq | Wk] stacked on the output dim, chunked along the contraction dim
    wqk = np.concatenate([np.asarray(Wq), np.asarray(Wk)], axis=1)
    wqk16 = np.ascontiguousarray(
        wqk.reshape(CC, 128, 2 * HEAD_SIZE), dtype=np.float16)
    Wv16 = np.ascontiguousarray(Wv, dtype=np.float16)
    ident, mask01 = _consts()

    in_maps = []
    for c in range(N_CORES):
        in_maps.append({
            "x": x16[c * B_SHARD:(c + 1) * B_SHARD],
            "Wqk": wqk16, "Wv": Wv16,
            "ident": ident, "mask01": mask01,
        })

    res = bass_utils.run_bass_kernel_spmd(
        nc, in_maps, core_ids=list(range(N_CORES)), trace=TRACE)
    LAST_RESULTS = res
    out = np.concatenate([res.results[c]["out"] for c in range(N_CORES)], axis=0)
    return np.ascontiguousarray(out, dtype=np.float32)


# revision 31
# speedup vs baseline: 1.1752x; 1.0623x over previous
"""Trainium2 Bass kernel for a causal single-head attention block.

Reference computation (fp32):
    q = x @ Wq; k = x @ Wk; v = x @ Wv        x: [B=256, T=256, C=384], W*: [384, 64]
    wei = softmax(causal_mask(q @ k.T / sqrt(C)))
    out = wei @ v                              out: [256, 256, 64]

Strategy: pure data parallel over B across 8 NeuronCores (32 batches/core).

All matmul operands are fp16: 2-byte operands stream through the PE at full
rate (4-byte fp32/fp32r streams at half rate), get fast weight load, and allow
the DMA engines' 2-byte transpose XBAR. PSUM accumulation stays fp32; measured
end-to-end error vs the fp32 reference is ~8e-4 (scale-relative).

Per-batch dataflow:
    xT    = DMA-transpose(x_b)                 [C, T] fp16 sbuf [128, 768], 3 DMAs,
                                               no PE/DVE involvement at all
    qT|kT = [Wq|Wk].T @ xT                     3 accumulating matmuls, M=128 packed
                                               (q on partitions 0-63, k on 64-127)
    kT    -> own tile via SBUF->SBUF DMA       (cross-partition move, scalar DGE)
    v     = xT.T @ Wv                          [128, 64+64] (both t tiles, one psum)
    sT    = kT[:, s_tile].T @ qT               scores transposed [s, t]
    P     = exp(sT * 1/sqrt(C)) * mask01       exp on ScalarE from psum, 0/1 mask
                                               multiply on DVE (fp16, 2x mode)
    oeT   = [v | 1].T @ P                      [65, 256] psum; row 64 = denominator Z
    out   = PE-transpose(oeT) * (1/Z)          normalize on ScalarE, DMA out (GpSimd
                                               DGE so the sync queue stays short)

Software pipeline (PE stream per iteration):
    sc(b) qkT(b+1) v(b+1) oe(b) finT(b-1)
so the softmax chain of batch b (ACT exp + DVE mask) runs under the next
batch's projections, and finT's oe-copy dependency is a full iteration old.
"""

import numpy as np

N_EMBED = 384
HEAD_SIZE = 64
H1 = HEAD_SIZE + 1
T = 256
B = 256
N_CORES = 8
B_SHARD = B // N_CORES  # 32
CC = N_EMBED // 128  # 3 contraction chunks
INV_SQRT_C = 1.0 / float(np.sqrt(N_EMBED))

_CACHE = {}

# test.py can flip these before calling kernel()
TRACE = False
LAST_RESULTS = None


def _build_program():
    import concourse.bacc as bacc
    import concourse.mybir as mybir
    import concourse.tile as tile
    from concourse import bass

    f32 = mybir.dt.float32
    f16 = mybir.dt.float16
    ts = bass.ts
    Exp = mybir.ActivationFunctionType.Exp
    Copy = mybir.ActivationFunctionType.Copy

    nc = bacc.Bacc("TRN2", target_bir_lowering=False, debug=False,
                   enable_asserts=False)

    x_d = nc.dram_tensor("x", [B_SHARD, T, N_EMBED], f16, kind="ExternalInput")
    wqk_d = nc.dram_tensor("Wqk", [CC, 128, 128], f16, kind="ExternalInput")
    wv_d = nc.dram_tensor("Wv", [N_EMBED, HEAD_SIZE], f16, kind="ExternalInput")
    ident_d = nc.dram_tensor("ident", [128, 128], f16, kind="ExternalInput")
    mask_d = nc.dram_tensor("mask01", [2, 128, T], f16, kind="ExternalInput")
    out_d = nc.dram_tensor("out", [B_SHARD, T, HEAD_SIZE], f32, kind="ExternalOutput")

    x_ap = x_d.ap()
    out_ap = out_d.ap()

    with tile.TileContext(nc) as tc:
        with (
            tc.tile_pool(name="const", bufs=1) as cpool,
            tc.tile_pool(name="xin", bufs=6) as xin_pool,
            tc.tile_pool(name="xt", bufs=2) as xt_pool,
            tc.tile_pool(name="proj", bufs=7) as proj_pool,
            tc.tile_pool(name="soft", bufs=6) as soft_pool,
            tc.tile_pool(name="outp", bufs=8) as out_pool,
            tc.tile_pool(name="ps", bufs=4, space="PSUM") as ps_pool,
            tc.tile_pool(name="ps16", bufs=4, space="PSUM") as ps16_pool,
        ):
            # ---- constants ----
            ident = cpool.tile([128, 128], f16, tag="ident")
            nc.sync.dma_start(ident[:], ident_d.ap())
            wqk_sb, wv_sb = [], []
            for cc in range(CC):
                t_ = cpool.tile([128, 128], f16, tag=f"wqk{cc}")
                nc.sync.dma_start(t_[:], wqk_d.ap()[cc])
                wqk_sb.append(t_)
                t_ = cpool.tile([128, HEAD_SIZE], f16, tag=f"wv{cc}")
                nc.sync.dma_start(t_[:], wv_d.ap()[ts(cc, 128), :])
                wv_sb.append(t_)
            mask_sb = []
            for st in range(2):
                t_ = cpool.tile([128, T], f16, tag=f"mask{st}")
                nc.sync.dma_start(t_[:], mask_d.ap()[st, :, :])
                mask_sb.append(t_)
            ones_col = cpool.tile([128, 1], f16, tag="ones")
            nc.gpsimd.memset(ones_col[:], 1.0)

            def load_x(b):
                tiles = []
                for tt in range(2):
                    t_ = xin_pool.tile([128, N_EMBED], f16, tag="x_nat")
                    nc.sync.dma_start(t_[:], x_ap[b, ts(tt, 128), :])
                    tiles.append(t_)
                return tiles

            def transpose_x(x_nat):
                """-> one [128, 768] fp16 sbuf tile; chunk cc at [:, cc*256:(cc+1)*256]"""
                ps = ps16_pool.tile([128, 3 * T], f16, tag="ps16")
                for cc in range(CC):
                    for tt in range(2):
                        nc.tensor.transpose(
                            ps[:, cc * T + tt * 128: cc * T + (tt + 1) * 128],
                            x_nat[tt][:, ts(cc, 128)], ident[:])
                sb = xt_pool.tile([128, 3 * T], f16, tag="xt")
                nc.vector.tensor_copy(sb[:], ps[:])
                return sb

            def qkT_mm(xt):
                """q and k projections in one M=128 stack; kT moved to its own
                base-0 tile with a cross-partition SBUF->SBUF DMA."""
                ps = ps_pool.tile([128, 512], f32, tag="ps")
                for cc in range(CC):
                    nc.tensor.matmul(ps[:, :T], wqk_sb[cc][:], xt[:, ts(cc, T)],
                                     start=(cc == 0), stop=(cc == CC - 1))
                sb = proj_pool.tile([128, T], f16, tag="qk")
                nc.vector.tensor_copy(sb[:], ps[:, :T])
                kT = proj_pool.tile([HEAD_SIZE, T], f16, tag="kT")
                nc.sync.dma_start(kT[:], sb[HEAD_SIZE:128, :])
                return sb, kT  # qT = sb[:64]

            def v_mm(xt):
                ps = ps_pool.tile([128, 512], f32, tag="ps")
                for tt in range(2):
                    for cc in range(CC):
                        nc.tensor.matmul(ps[:, tt * 128:tt * 128 + HEAD_SIZE],
                                         xt[:, cc * T + tt * 128: cc * T + (tt + 1) * 128],
                                         wv_sb[cc][:],
                                         start=(cc == 0), stop=(cc == CC - 1))
                v_ext = []
                for tt in range(2):
                    sb = proj_pool.tile([128, H1], f16, tag="v_ext")
                    nc.vector.tensor_copy(sb[:, :HEAD_SIZE],
                                          ps[:, tt * 128:tt * 128 + HEAD_SIZE])
                    nc.vector.tensor_copy(sb[:, HEAD_SIZE:H1], ones_col[:])
                    v_ext.append(sb)
                return v_ext

            def scores_mm(qk, kT):
                ps = ps_pool.tile([128, 512], f32, tag="ps")
                for st in range(2):
                    nc.tensor.matmul(ps[:, ts(st, T)], kT[:, ts(st, 128)],
                                     qk[:HEAD_SIZE, :], start=True, stop=True)
                return ps  # scoresT: s_tile st at [:, st*256:(st+1)*256]

            def softmax(sc_ps):
                """P [128, 256] fp16 per s_tile: exp then 0/1-mask, unnormalized."""
                p_sb = []
                for st in range(2):
                    e = soft_pool.tile([128, T], f16, tag=f"e{st}")
                    nc.scalar.activation(e[:], sc_ps[:, ts(st, T)], Exp,
                                         scale=INV_SQRT_C)
                    p = soft_pool.tile([128, T], f16, tag=f"p{st}")
                    nc.vector.tensor_mul(p[:], e[:], mask_sb[st][:])
                    p_sb.append(p)
                return p_sb

            def oe_mm(v_ext, p_sb):
                ps = ps_pool.tile([128, 512], f32, tag="ps")
                for st in range(2):
                    nc.tensor.matmul(ps[:H1, :T], v_ext[st][:], p_sb[st][:],
                                     start=(st == 0), stop=(st == 1))
                sb = out_pool.tile([H1 + 1, T], f16, tag="oe")
                # row 65 is never written by the matmul and never read after the
                # transpose; copying 66 rows keeps partition bases aligned
                nc.vector.tensor_copy(sb[:], ps[:H1 + 1, :T])
                return sb

            def fin_mm(oe):
                ps = ps16_pool.tile([128, 512], f16, tag="ps16")
                for tt in range(2):
                    nc.tensor.transpose(ps[:, tt * 128:tt * 128 + H1 + 1],
                                        oe[:, ts(tt, 128)],
                                        ident[:H1 + 1, :H1 + 1])
                return ps

            def norm_store(b, fin_ps):
                fps = fin_ps[:]
                rz = out_pool.tile([128, 2], f32, tag="rz")
                # both Z columns (offsets 64 and 192) in one strided reciprocal
                nc.vector.reciprocal(rz[:], fps[:, HEAD_SIZE:256:128])
                for tt in range(2):
                    o = out_pool.tile([128, HEAD_SIZE], f32, tag="o")
                    nc.scalar.activation(o[:], fps[:, tt * 128: tt * 128 + HEAD_SIZE],
                                         Copy, scale=rz[:, tt:tt + 1])
                    nc.gpsimd.dma_start(out_ap[b, ts(tt, 128), :], o[:])

            # ---- software-pipelined batch loop ----
            # Projections run TWO iterations ahead of their scores matmul so the
            # cross-partition kT DMA has a full iteration of slack.
            # PE stream per iteration:
            #   sc(b) T(b+2) qkT(b+2) v(b+2) oe(b) finT(b-1)
            x_nat = [None] * B_SHARD
            qks, kTs, vs = {}, {}, {}
            x_nat[0] = load_x(0)
            x_nat[1] = load_x(1)
            if B_SHARD > 2:
                x_nat[2] = load_x(2)
            for j in range(min(2, B_SHARD)):
                xtj = transpose_x(x_nat[j])
                qks[j], kTs[j] = qkT_mm(xtj)
                vs[j] = v_mm(xtj)
            prev_oe = None
            for b in range(B_SHARD):
                if b + 3 < B_SHARD:
                    x_nat[b + 3] = load_x(b + 3)
                sc_ps = scores_mm(qks.pop(b), kTs.pop(b))
                p_sb = softmax(sc_ps)
                if b + 2 < B_SHARD:
                    xtn = transpose_x(x_nat[b + 2])
                    qks[b + 2], kTs[b + 2] = qkT_mm(xtn)
                    vs[b + 2] = v_mm(xtn)
                oe_p = prev_oe
                prev_oe = oe_mm(vs.pop(b), p_sb)
                if oe_p is not None:
                    norm_store(b - 1, fin_mm(oe_p))
            norm_store(B_SHARD - 1, fin_mm(prev_oe))

    nc.compile()
    return nc


def _consts():
    ident = np.eye(128, dtype=np.float16)
    # mask01[st][s_local, t] = 0 where global s > t (causal), else 1
    s = np.arange(T)[:, None]
    t = np.arange(T)[None, :]
    full = (s <= t).astype(np.float16)
    mask01 = np.stack([full[:128], full[128:]], axis=0)
    return ident, mask01


def _spot_check(out, x, Wq, Wk, Wv, batches):
    """Numpy reference for a few batches -- guards against transient device
    flakiness. The fp16 kernel's error is ~3e-3 abs; garbage is ~1e0."""
    for b in batches:
        xb = np.asarray(x[b], dtype=np.float32)
        q = xb @ Wq
        k = xb @ Wk
        v = xb @ Wv
        s = (q @ k.T) * np.float32(INV_SQRT_C)
        tmask = np.tril(np.ones((T, T), dtype=bool))
        s = np.where(tmask, s, -np.inf)
        w = np.exp(s - s.max(axis=-1, keepdims=True))
        o = (w @ v) / w.sum(axis=-1, keepdims=True)
        if np.max(np.abs(out[b] - o)) > 0.05 * max(np.max(np.abs(o)), 1e-3):
            return False
    return True


def kernel(x, Wq, Wk, Wv):
    global LAST_RESULTS
    from concourse import bass_utils

    if "nc" not in _CACHE:
        _CACHE["nc"] = _build_program()
    nc = _CACHE["nc"]

    x16 = np.ascontiguousarray(x, dtype=np.float16)
    # [Wq | Wk] stacked on the output dim, chunked along the contraction dim
    wqk = np.concatenate([np.asarray(Wq), np.asarray(Wk)], axis=1)
    wqk16 = np.ascontiguousarray(
        wqk.reshape(CC, 128, 2 * HEAD_SIZE), dtype=np.float16)
    Wv16 = np.ascontiguousarray(Wv, dtype=np.float16)
    ident, mask01 = _consts()

    in_maps = []
    for c in range(N_CORES):
        in_maps.append({
            "x": x16[c * B_SHARD:(c + 1) * B_SHARD],
            "Wqk": wqk16, "Wv": Wv16,
            "ident": ident, "mask01": mask01,
        })

    xf = np.ascontiguousarray(x, dtype=np.float32)
    Wqf = np.asarray(Wq, dtype=np.float32)
    Wkf = np.asarray(Wk, dtype=np.float32)
    Wvf = np.asarray(Wv, dtype=np.float32)
    check_batches = [c * B_SHARD for c in range(N_CORES)]
    for attempt in range(3):
        res = bass_utils.run_bass_kernel_spmd(
            nc, in_maps, core_ids=list(range(N_CORES)), trace=TRACE)
        LAST_RESULTS = res
        out = np.concatenate([res.results[c]["out"] for c in range(N_CORES)],
                             axis=0)
        out = np.ascontiguousarray(out, dtype=np.float32)
        if _spot_check(out, xf, Wqf, Wkf, Wvf, check_batches):
            return out
    return out


# revision 34
# speedup vs baseline: 1.1978x; 1.0193x over previous
"""Trainium2 Bass kernel for a causal single-head attention block.

Reference computation (fp32):
    q = x @ Wq; k = x @ Wk; v = x @ Wv        x: [B=256, T=256, C=384], W*: [384, 64]
    wei = softmax(causal_mask(q @ k.T / sqrt(C)))
    out = wei @ v                              out: [256, 256, 64]

Strategy: pure data parallel over B across 8 NeuronCores (32 batches/core).

All matmul operands are fp16: 2-byte operands stream through the PE at full
rate (4-byte fp32/fp32r streams at half rate), get fast weight load, and allow
the DMA engines' 2-byte transpose XBAR. PSUM accumulation stays fp32; measured
end-to-end error vs the fp32 reference is ~8e-4 (scale-relative).

Per-batch dataflow:
    xT    = DMA-transpose(x_b)                 [C, T] fp16 sbuf [128, 768], 3 DMAs,
                                               no PE/DVE involvement at all
    qT|kT = [Wq|Wk].T @ xT                     3 accumulating matmuls, M=128 packed
                                               (q on partitions 0-63, k on 64-127)
    kT    -> own tile via SBUF->SBUF DMA       (cross-partition move, scalar DGE)
    v     = xT.T @ Wv                          [128, 64+64] (both t tiles, one psum)
    sT    = kT[:, s_tile].T @ qT               scores transposed [s, t]
    P     = exp(sT * 1/sqrt(C)) * mask01       exp on ScalarE from psum, 0/1 mask
                                               multiply on DVE (fp16, 2x mode)
    oeT   = [v | 1].T @ P                      [65, 256] psum; row 64 = denominator Z
    out   = PE-transpose(oeT) * (1/Z)          normalize on ScalarE, DMA out (GpSimd
                                               DGE so the sync queue stays short)

Software pipeline (PE stream per iteration):
    sc(b) qkT(b+1) v(b+1) oe(b) finT(b-1)
so the softmax chain of batch b (ACT exp + DVE mask) runs under the next
batch's projections, and finT's oe-copy dependency is a full iteration old.
"""

import numpy as np

N_EMBED = 384
HEAD_SIZE = 64
H1 = HEAD_SIZE + 1
T = 256
B = 256
N_CORES = 8
B_SHARD = B // N_CORES  # 32
CC = N_EMBED // 128  # 3 contraction chunks
INV_SQRT_C = 1.0 / float(np.sqrt(N_EMBED))

_CACHE = {}

# test.py can flip these before calling kernel()
TRACE = False
LAST_RESULTS = None


def _build_program():
    import concourse.bacc as bacc
    import concourse.mybir as mybir
    import concourse.tile as tile
    from concourse import bass

    f32 = mybir.dt.float32
    f16 = mybir.dt.float16
    ts = bass.ts
    Exp = mybir.ActivationFunctionType.Exp
    Copy = mybir.ActivationFunctionType.Copy

    nc = bacc.Bacc("TRN2", target_bir_lowering=False, debug=False,
                   enable_asserts=False)

    x_d = nc.dram_tensor("x", [B_SHARD, T, N_EMBED], f16, kind="ExternalInput")
    wqk_d = nc.dram_tensor("Wqk", [CC, 128, 128], f16, kind="ExternalInput")
    wv_d = nc.dram_tensor("Wv", [N_EMBED, HEAD_SIZE], f16, kind="ExternalInput")
    ident_d = nc.dram_tensor("ident", [128, 128], f16, kind="ExternalInput")
    mask_d = nc.dram_tensor("mask01", [2, 128, T], f16, kind="ExternalInput")
    out_d = nc.dram_tensor("out", [B_SHARD, T, HEAD_SIZE], f32, kind="ExternalOutput")

    x_ap = x_d.ap()
    out_ap = out_d.ap()

    with tile.TileContext(nc) as tc:
        with (
            tc.tile_pool(name="const", bufs=1) as cpool,
            tc.tile_pool(name="xin", bufs=6) as xin_pool,
            tc.tile_pool(name="xt", bufs=2) as xt_pool,
            tc.tile_pool(name="proj", bufs=7) as proj_pool,
            tc.tile_pool(name="soft", bufs=6) as soft_pool,
            tc.tile_pool(name="outp", bufs=8) as out_pool,
            tc.tile_pool(name="ps", bufs=4, space="PSUM") as ps_pool,
            tc.tile_pool(name="ps16", bufs=4, space="PSUM") as ps16_pool,
        ):
            # ---- constants ----
            ident = cpool.tile([128, 128], f16, tag="ident")
            nc.sync.dma_start(ident[:], ident_d.ap())
            wqk_sb, wv_sb = [], []
            for cc in range(CC):
                t_ = cpool.tile([128, 128], f16, tag=f"wqk{cc}")
                nc.sync.dma_start(t_[:], wqk_d.ap()[cc])
                wqk_sb.append(t_)
                t_ = cpool.tile([128, HEAD_SIZE], f16, tag=f"wv{cc}")
                nc.sync.dma_start(t_[:], wv_d.ap()[ts(cc, 128), :])
                wv_sb.append(t_)
            mask_sb = []
            for st in range(2):
                t_ = cpool.tile([128, T], f16, tag=f"mask{st}")
                nc.sync.dma_start(t_[:], mask_d.ap()[st, :, :])
                mask_sb.append(t_)
            ones_col = cpool.tile([128, 1], f16, tag="ones")
            nc.gpsimd.memset(ones_col[:], 1.0)

            def load_x(b):
                # one DMA for the whole batch: [256, 384] -> [128, 2*384]
                t_ = xin_pool.tile([128, 2 * N_EMBED], f16, tag="x_nat")
                nc.sync.dma_start(
                    t_[:].rearrange("p (tt c) -> p tt c", tt=2),
                    x_ap[b].rearrange("(tt p) c -> p tt c", tt=2))
                return t_

            def transpose_x(x_nat):
                """-> one [128, 768] fp16 sbuf tile; chunk cc at [:, cc*256:(cc+1)*256]"""
                ps = ps16_pool.tile([128, 3 * T], f16, tag="ps16")
                for cc in range(CC):
                    for tt in range(2):
                        nc.tensor.transpose(
                            ps[:, cc * T + tt * 128: cc * T + (tt + 1) * 128],
                            x_nat[:, tt * N_EMBED + cc * 128:
                                  tt * N_EMBED + (cc + 1) * 128], ident[:])
                sb = xt_pool.tile([128, 3 * T], f16, tag="xt")
                nc.vector.tensor_copy(sb[:], ps[:])
                return sb

            def qkT_mm(xt):
                """q and k projections in one M=128 stack; kT moved to its own
                base-0 tile with a cross-partition SBUF->SBUF DMA."""
                ps = ps_pool.tile([128, 512], f32, tag="ps")
                for cc in range(CC):
                    nc.tensor.matmul(ps[:, :T], wqk_sb[cc][:], xt[:, ts(cc, T)],
                                     start=(cc == 0), stop=(cc == CC - 1))
                sb = proj_pool.tile([128, T], f16, tag="qk")
                nc.vector.tensor_copy(sb[:], ps[:, :T])
                kT = proj_pool.tile([HEAD_SIZE, T], f16, tag="kT")
                nc.sync.dma_start(kT[:], sb[HEAD_SIZE:128, :])
                return sb, kT  # qT = sb[:64]

            def v_mm(xt):
                ps = ps_pool.tile([128, 512], f32, tag="ps")
                for tt in range(2):
                    for cc in range(CC):
                        nc.tensor.matmul(ps[:, tt * 128:tt * 128 + HEAD_SIZE],
                                         xt[:, cc * T + tt * 128: cc * T + (tt + 1) * 128],
                                         wv_sb[cc][:],
                                         start=(cc == 0), stop=(cc == CC - 1))
                v_ext = []
                for tt in range(2):
                    sb = proj_pool.tile([128, H1], f16, tag="v_ext")
                    nc.vector.tensor_copy(sb[:, :HEAD_SIZE],
                                          ps[:, tt * 128:tt * 128 + HEAD_SIZE])
                    nc.vector.tensor_copy(sb[:, HEAD_SIZE:H1], ones_col[:])
                    v_ext.append(sb)
                return v_ext

            def scores_mm(qk, kT):
                ps = ps_pool.tile([128, 512], f32, tag="ps")
                for st in range(2):
                    nc.tensor.matmul(ps[:, ts(st, T)], kT[:, ts(st, 128)],
                                     qk[:HEAD_SIZE, :], start=True, stop=True)
                return ps  # scoresT: s_tile st at [:, st*256:(st+1)*256]

            def softmax(sc_ps):
                """P [128, 256] fp16 per s_tile: exp then 0/1-mask, unnormalized."""
                p_sb = []
                for st in range(2):
                    e = soft_pool.tile([128, T], f16, tag=f"e{st}")
                    nc.scalar.activation(e[:], sc_ps[:, ts(st, T)], Exp,
                                         scale=INV_SQRT_C)
                    p = soft_pool.tile([128, T], f16, tag=f"p{st}")
                    nc.vector.tensor_mul(p[:], e[:], mask_sb[st][:])
                    p_sb.append(p)
                return p_sb

            def oe_mm(v_ext, p_sb):
                ps = ps_pool.tile([128, 512], f32, tag="ps")
                for st in range(2):
                    nc.tensor.matmul(ps[:H1, :T], v_ext[st][:], p_sb[st][:],
                                     start=(st == 0), stop=(st == 1))
                sb = out_pool.tile([H1 + 1, T], f16, tag="oe")
                # row 65 is never written by the matmul and never read after the
                # transpose; copying 66 rows keeps partition bases aligned
                nc.vector.tensor_copy(sb[:], ps[:H1 + 1, :T])
                return sb

            def fin_mm(oe):
                ps = ps16_pool.tile([128, 512], f16, tag="ps16")
                for tt in range(2):
                    nc.tensor.transpose(ps[:, tt * 128:tt * 128 + H1 + 1],
                                        oe[:, ts(tt, 128)],
                                        ident[:H1 + 1, :H1 + 1])
                return ps

            def norm_store(b, fin_ps):
                fps = fin_ps[:]
                rz = out_pool.tile([128, 2], f32, tag="rz")
                # both Z columns (offsets 64 and 192) in one strided reciprocal
                nc.vector.reciprocal(rz[:], fps[:, HEAD_SIZE:256:128])
                for tt in range(2):
                    o = out_pool.tile([128, HEAD_SIZE], f32, tag="o")
                    nc.scalar.activation(o[:], fps[:, tt * 128: tt * 128 + HEAD_SIZE],
                                         Copy, scale=rz[:, tt:tt + 1])
                    nc.gpsimd.dma_start(out_ap[b, ts(tt, 128), :], o[:])

            # ---- software-pipelined batch loop ----
            # Projections run TWO iterations ahead of their scores matmul so the
            # cross-partition kT DMA has a full iteration of slack.
            # PE stream per iteration:
            #   sc(b) T(b+2) qkT(b+2) v(b+2) oe(b) finT(b-1)
            x_nat = [None] * B_SHARD
            qks, kTs, vs = {}, {}, {}
            for j in range(min(4, B_SHARD)):
                x_nat[j] = load_x(j)
            for j in range(min(2, B_SHARD)):
                xtj = transpose_x(x_nat[j])
                qks[j], kTs[j] = qkT_mm(xtj)
                vs[j] = v_mm(xtj)
            prev_oe = None
            for b in range(B_SHARD):
                if b + 4 < B_SHARD:
                    x_nat[b + 4] = load_x(b + 4)
                sc_ps = scores_mm(qks.pop(b), kTs.pop(b))
                p_sb = softmax(sc_ps)
                if b + 2 < B_SHARD:
                    xtn = transpose_x(x_nat[b + 2])
                    qks[b + 2], kTs[b + 2] = qkT_mm(xtn)
                    vs[b + 2] = v_mm(xtn)
                oe_p = prev_oe
                prev_oe = oe_mm(vs.pop(b), p_sb)
                if oe_p is not None:
                    norm_store(b - 1, fin_mm(oe_p))
            norm_store(B_SHARD - 1, fin_mm(prev_oe))

    nc.compile()
    return nc


def _consts():
    ident = np.eye(128, dtype=np.float16)
    # mask01[st][s_local, t] = 0 where global s > t (causal), else 1
    s = np.arange(T)[:, None]
    t = np.arange(T)[None, :]
    full = (s <= t).astype(np.float16)
    mask01 = np.stack([full[:128], full[128:]], axis=0)
    return ident, mask01


def kernel(x, Wq, Wk, Wv):
    global LAST_RESULTS
    from concourse import bass_utils

    if "nc" not in _CACHE:
        _CACHE["nc"] = _build_program()
    nc = _CACHE["nc"]

    x16 = np.ascontiguousarray(x, dtype=np.float16)
    # [Wq | Wk] stacked on the output dim, chunked along the contraction dim
    wqk = np.concatenate([np.asarray(Wq), np.asarray(Wk)], axis=1)
    wqk16 = np.ascontiguousarray(
        wqk.reshape(CC, 128, 2 * HEAD_SIZE), dtype=np.float16)
    Wv16 = np.ascontiguousarray(Wv, dtype=np.float16)
    ident, mask01 = _consts()

    in_maps = []
    for c in range(N_CORES):
        in_maps.append({
            "x": x16[c * B_SHARD:(c + 1) * B_SHARD],
            "Wqk": wqk16, "Wv": Wv16,
            "ident": ident, "mask01": mask01,
        })

    res = bass_utils.run_bass_kernel_spmd(
        nc, in_maps, core_ids=list(range(N_CORES)), trace=TRACE)
    LAST_RESULTS = res
    out = np.concatenate([res.results[c]["out"] for c in range(N_CORES)], axis=0)
    return np.ascontiguousarray(out, dtype=np.float32)
